# revision 22
# baseline (speedup 1.0000x reference)
"""CoarseMatching (LoFTR-style dual-softmax matching) on 8 Trainium2 cores.

Sharding: core c handles batch n = c//2, L-half h = c%2 (rows [h*2400, (h+1)*2400)).
Each core computes its sim shard (matmul), conf shard (dual softmax), and the
host derives the small outputs (mutual-NN / threshold / keypoints) from conf.

conf[l,s] = exp(2*st - 2C) / (R'_l * S'_s)   with st = 10*sim, C = 45,
R'_l = sum_s exp(st - C), S'_s = sum_l exp(st - C)  (softmax shift cancels
exactly; C keeps exp() in fp32 range).

Matmul precision: "bf16x3" splits each f32 feature into hi=bf16(x) and
lo=bf16(x-hi); sim = hi.hi + hi.lo + lo.hi accumulated in f32 PSUM gives
~1e-5 relative accuracy at 3x bf16 rate (vs 4x slower native fp32).

Pipeline (3 device programs, data stays on-device between them):
  A: sim matmul -> sim out; E = exp(10*sim - C); row sums -> on-core
     rbias = -(2C + log R'); column sums tree-reduced to [1, S].
  glue: pure lax.psum of the [1, S] column-sum partials across the 2 cores
     of each batch (the only cross-core step).
  B: conf = exp(20*sim - (2C + log R')) * (1/S'); sim comes from a mix of
     HBM re-reads (first NREAD blocks) and matmul recompute (rest) to
     balance the DMA and TensorE pipes.
"""

import sys
import contextlib

import numpy as np
import ml_dtypes

sys.path.insert(0, "/opt/trn_rl_repo")

import concourse.bass as bass  # noqa: E402
import concourse.tile as tile  # noqa: E402
from concourse import bacc, mybir  # noqa: E402

F32 = mybir.dt.float32
BF16 = mybir.dt.bfloat16

# Problem geometry (hardcoded per contract)
N, L, S, C = 4, 4800, 4800, 256
LH = L // 2                  # 2400 rows per core
NBLK = (LH + 127) // 128     # 19 row blocks (last one 96 rows)
SCHUNK = 480                 # psum chunk (one bank holds 512 f32)
NCHUNK = S // SCHUNK         # 10
TEMP = 0.1
THR = 0.2
BORDER = 2
SHIFT = 45.0                 # constant softmax shift C

MM_DTYPE = "bf16x3"          # "fp32" | "bf16x3"
NREAD = 6                    # B: blocks whose sim is re-read from HBM

_CACHE = {}


def _declare_feats(nc):
    if MM_DTYPE == "fp32":
        names = [("f0t", [C, LH], F32), ("f1t", [C, S], F32)]
    else:
        names = [("f0h", [C, LH], BF16), ("f0l", [C, LH], BF16),
                 ("f1h", [C, S], BF16), ("f1l", [C, S], BF16)]
    return [nc.dram_tensor(n, sh, dt, kind="ExternalInput").ap()
            for (n, sh, dt) in names]


def _load_feats(nc, pool, feat_aps):
    tiles = []
    for i, ap in enumerate(feat_aps):
        free = ap.shape[1]
        t = pool.tile([128, 2, free], ap.dtype, tag=f"feat{i}")
        nc.sync.dma_start(t[:], ap.rearrange("(c k) f -> k c f", k=128))
        tiles.append(t)
    if MM_DTYPE == "fp32":
        return [tiles[0]], [tiles[1]], [(0, 0)]
    return [tiles[0], tiles[1]], [tiles[2], tiles[3]], [(0, 0), (0, 1), (1, 0)]


def _emit_matmul(nc, ps, lhs_parts, rhs_parts, mm_groups, b, pb, sj):
    lb = bass.ds(b * 128, pb)
    n_terms = len(mm_groups) * 2
    t = 0
    for (li, ri) in mm_groups:
        for cc in range(2):
            t += 1
            nc.tensor.matmul(
                ps[:pb, :],
                lhs_parts[li][:, cc, lb],
                rhs_parts[ri][:, cc, sj],
                start=(t == 1),
                stop=(t == n_terms),
            )


def _build_a():
    nc = bacc.Bacc("TRN2", target_bir_lowering=False, debug=False)
    feat_aps = _declare_feats(nc)
    sim_out = nc.dram_tensor("sim", [LH, S], F32, kind="ExternalOutput").ap()
    colsum_out = nc.dram_tensor("colsum", [1, S], F32, kind="ExternalOutput").ap()
    rbias_out = nc.dram_tensor("rbias", [128, NBLK], F32, kind="ExternalOutput").ap()

    with tile.TileContext(nc) as tc, contextlib.ExitStack() as ctx:
        const_pool = ctx.enter_context(tc.tile_pool(name="const", bufs=1))
        psum = ctx.enter_context(tc.tile_pool(name="ps", bufs=4, space="PSUM"))
        simp = ctx.enter_context(tc.tile_pool(name="simp", bufs=3))
        ep = ctx.enter_context(tc.tile_pool(name="ep", bufs=2))
        accp = ctx.enter_context(tc.tile_pool(name="accp", bufs=1))

        lhs, rhs, mm_groups = _load_feats(nc, const_pool, feat_aps)

        acc = accp.tile([128, S], F32)
        nc.vector.memset(acc[:], 0.0)
        rstile = accp.tile([128, NBLK], F32)
        nc.vector.memset(rstile[:], 1.0)   # pad rows -> log(1) = 0
        nbias = accp.tile([128, 1], F32)
        nc.vector.memset(nbias[:], -SHIFT)

        for b in range(NBLK):
            pb = min(128, LH - b * 128)
            sim_sb = simp.tile([128, S], F32, tag="sim")
            e_sb = ep.tile([128, S], F32, tag="e")
            racc = ep.tile([128, NCHUNK], F32, tag="racc")
            for j in range(NCHUNK):
                ps = psum.tile([128, SCHUNK], F32, tag="mm")
                sj = bass.ts(j, SCHUNK)
                _emit_matmul(nc, ps, lhs, rhs, mm_groups, b, pb, sj)
                # raw sim copy (psum -> sbuf); nc.any lets Tile pick DVE/ACT
                nc.any.tensor_copy(sim_sb[:pb, sj], ps[:pb, :])
                # E = exp(10*sim - C), row-accumulated per chunk
                nc.scalar.activation(
                    e_sb[:pb, sj], ps[:pb, :],
                    mybir.ActivationFunctionType.Exp,
                    bias=nbias[:pb, 0:1], scale=10.0,
                    accum_out=racc[:pb, j:j + 1],
                )
            nc.sync.dma_start(sim_out[b * 128 : b * 128 + pb, :], sim_sb[:pb, :])
            nc.vector.tensor_add(acc[:pb, :], acc[:pb, :], e_sb[:pb, :])
            nc.vector.reduce_sum(rstile[:pb, b:b + 1], racc[:pb, :],
                                 axis=mybir.AxisListType.X)

        # column-sum partial: contract the 128 partitions with a ones-vector
        # matmul (DVE lanes cannot cross partitions)
        ones = accp.tile([128, 1], F32)
        nc.vector.memset(ones[:], 1.0)
        acc_red = accp.tile([1, S], F32)
        for j in range(NCHUNK):
            pr = psum.tile([1, SCHUNK], F32, tag="red")
            sj = bass.ts(j, SCHUNK)
            nc.tensor.matmul(pr[:, :], ones[:], acc[:, sj],
                             start=True, stop=True)
            nc.any.tensor_copy(acc_red[0:1, sj], pr[:, :])
        nc.sync.dma_start(colsum_out[:], acc_red[0:1, :])

        # rbias = -(2C + log R')
        logr = accp.tile([128, NBLK], F32)
        nc.scalar.activation(logr[:], rstile[:],
                             mybir.ActivationFunctionType.Ln)
        rbias_t = accp.tile([128, NBLK], F32)
        nc.scalar.activation(rbias_t[:], logr[:],
                             mybir.ActivationFunctionType.Copy,
                             bias=-2.0 * SHIFT, scale=-1.0)
        nc.sync.dma_start(rbias_out[:], rbias_t[:])

    nc.compile()
    return nc


def _build_b():
    nc = bacc.Bacc("TRN2", target_bir_lowering=False, debug=False)
    feat_aps = _declare_feats(nc)
    stot = nc.dram_tensor("stot", [1, S], F32, kind="ExternalInput").ap()
    rbias = nc.dram_tensor("rbias", [128, NBLK], F32, kind="ExternalInput").ap()
    sim_in = nc.dram_tensor("sim_in", [LH, S], F32, kind="ExternalInput").ap()
    conf_out = nc.dram_tensor("conf", [LH, S], F32, kind="ExternalOutput").ap()
    invs_dram = nc.dram_tensor("invs_bounce", [1, S], F32)

    with tile.TileContext(nc) as tc, contextlib.ExitStack() as ctx:
        const_pool = ctx.enter_context(tc.tile_pool(name="const", bufs=1))
        psum = ctx.enter_context(tc.tile_pool(name="ps", bufs=4, space="PSUM"))
        xp = ctx.enter_context(tc.tile_pool(name="xp", bufs=2))
        sinp = ctx.enter_context(tc.tile_pool(name="sinp", bufs=2))

        lhs, rhs, mm_groups = _load_feats(nc, const_pool, feat_aps)

        # invs = 1 / S'  broadcast to all 128 partitions via a DRAM bounce
        stot_sb = const_pool.tile([1, S], F32)
        nc.sync.dma_start(stot_sb[:], stot[:])
        nc.vector.reciprocal(stot_sb[:], stot_sb[:])
        nc.sync.dma_start(invs_dram[:], stot_sb[:])
        invs_b = const_pool.tile([128, S], F32)
        nc.sync.dma_start(invs_b[:], invs_dram[0:1, :].broadcast_to((128, S)))

        rbias_sb = const_pool.tile([128, NBLK], F32)
        nc.sync.dma_start(rbias_sb[:], rbias[:])

        for b in range(NBLK):
            pb = min(128, LH - b * 128)
            x_sb = xp.tile([128, S], F32, tag="x")
            if b < NREAD:
                sim_sb = sinp.tile([128, S], F32, tag="sin")
                nc.sync.dma_start(sim_sb[:pb, :],
                                  sim_in[b * 128 : b * 128 + pb, :])
                nc.scalar.activation(
                    x_sb[:pb, :], sim_sb[:pb, :],
                    mybir.ActivationFunctionType.Exp,
                    bias=rbias_sb[:pb, b:b + 1], scale=20.0,
                )
            else:
                for j in range(NCHUNK):
                    ps = psum.tile([128, SCHUNK], F32, tag="mm")
                    sj = bass.ts(j, SCHUNK)
                    _emit_matmul(nc, ps, lhs, rhs, mm_groups, b, pb, sj)
                    nc.scalar.activation(
                        x_sb[:pb, sj], ps[:pb, :],
                        mybir.ActivationFunctionType.Exp,
                        bias=rbias_sb[:pb, b:b + 1], scale=20.0,
                    )
            nc.vector.tensor_mul(x_sb[:pb, :], x_sb[:pb, :], invs_b[:pb, :])
            nc.sync.dma_start(conf_out[b * 128 : b * 128 + pb, :], x_sb[:pb, :])

    nc.compile()
    return nc


def _get_kernels():
    if "a" not in _CACHE:
        _CACHE["a"] = _build_a()
        _CACHE["b"] = _build_b()
    return _CACHE["a"], _CACHE["b"]


def _bir_spec(nc):
    """(input names, output names, output avals, partition name)."""
    import jax
    pname = nc.partition_id_tensor.name if nc.partition_id_tensor else None
    in_names, out_names, out_avals = [], [], []
    for alloc in nc.m.functions[0].allocations:
        if not isinstance(alloc, mybir.MemoryLocationSet):
            continue
        name = alloc.memorylocations[0].name
        if alloc.kind == "ExternalInput":
            if name != pname:
                in_names.append(name)
        elif alloc.kind == "ExternalOutput":
            shape = tuple(alloc.tensor_shape)
            dtype = mybir.dt.np(alloc.dtype)
            out_names.append(name)
            out_avals.append(jax.core.ShapedArray(shape, dtype))
    return in_names, out_names, out_avals, pname


def _bind_bass(nc, spec, operands):
    """operands: inputs in allocation order, then zero output buffers."""
    from concourse.bass2jax import _bass_exec_p, partition_id_tensor
    in_names, out_names, out_avals, pname = spec
    all_names = tuple(in_names) + tuple(out_names)
    operands = list(operands)
    if pname is not None:
        operands.append(partition_id_tensor())
        all_names = all_names + (pname,)
    outs = _bass_exec_p.bind(
        *operands,
        out_avals=tuple(out_avals),
        in_names=all_names,
        out_names=tuple(out_names),
        lowering_input_output_aliases=(),
        sim_require_finite=True,
        sim_require_nnan=True,
        nc=nc,
    )
    return dict(zip(out_names, outs))


def _get_runner():
    """Three jitted SPMD programs: launch A, psum glue, launch B."""
    if "runner" in _CACHE:
        return _CACHE["runner"]
    import jax
    import jax.numpy as jnp
    from jax.sharding import Mesh, PartitionSpec, NamedSharding
    from jax.experimental.shard_map import shard_map
    from concourse.bass2jax import install_neuronx_cc_hook

    install_neuronx_cc_hook()
    nc_a, nc_b = _get_kernels()
    spec_a, spec_b = _bir_spec(nc_a), _bir_spec(nc_b)

    devices = np.asarray(jax.devices()[:8]).reshape(4, 2)
    mesh = Mesh(devices, ("batch", "half"))
    pcore = PartitionSpec(("batch", "half"))
    shard = NamedSharding(mesh, pcore)

    feat_names = (["f0t", "f1t"] if MM_DTYPE == "fp32"
                  else ["f0h", "f0l", "f1h", "f1l"])
    nf = len(feat_names)
    assert list(spec_a[0]) == feat_names, spec_a[0]
    assert list(spec_b[0]) == feat_names + ["stot", "rbias", "sim_in"], spec_b[0]

    def body_a(*ops):
        outs = _bind_bass(nc_a, spec_a, ops)
        return outs["sim"], outs["colsum"], outs["rbias"]

    def body_glue(colsum):
        return jax.lax.psum(colsum, "half")

    def body_b(*ops):
        outs = _bind_bass(nc_b, spec_b, ops)
        return (outs["conf"],)

    n_a = nf + len(spec_a[1])
    n_b = nf + 3 + len(spec_b[1])
    fn_a = jax.jit(
        shard_map(body_a, mesh=mesh, in_specs=(pcore,) * n_a,
                  out_specs=(pcore,) * 3, check_rep=False))
    fn_glue = jax.jit(
        shard_map(body_glue, mesh=mesh, in_specs=(pcore,),
                  out_specs=pcore, check_rep=False))
    fn_b = jax.jit(
        shard_map(body_b, mesh=mesh, in_specs=(pcore,) * n_b,
                  out_specs=(pcore,), check_rep=False))

    def zeros_for(spec):
        _, _, out_avals, _ = spec
        return [jnp.zeros((8 * a.shape[0],) + tuple(a.shape[1:]), a.dtype)
                for a in out_avals]

    fn_zeros_a = jax.jit(lambda: tuple(zeros_for(spec_a)),
                         out_shardings=(shard,) * len(spec_a[1]))
    fn_zeros_b = jax.jit(lambda: tuple(zeros_for(spec_b)),
                         out_shardings=(shard,) * len(spec_b[1]))

    def run(staged_feats):
        if "zeros" not in _CACHE:
            _CACHE["zeros"] = (fn_zeros_a(), fn_zeros_b())
        za, zb = _CACHE["zeros"]
        sim, colsum, rbias = fn_a(*staged_feats, *za)
        stot = fn_glue(colsum)
        (conf,) = fn_b(*staged_feats, stot, rbias, sim, *zb)
        return sim, conf

    _CACHE["runner"] = (run, feat_names, [shard] * nf)
    return _CACHE["runner"]


def _border_valid(n):
    idx = np.arange(n)
    return (idx >= BORDER) & (idx < n - BORDER)


def _feat_inputs(feat_c0, feat_c1, core):
    # reference pre-scales each feature by 1/sqrt(C) = 1/16 (exact in fp32)
    n, h = core // 2, core % 2
    f0t = np.ascontiguousarray(feat_c0[n, h * LH : (h + 1) * LH].T) * np.float32(0.0625)
    f1t = np.ascontiguousarray(feat_c1[n].T) * np.float32(0.0625)
    if MM_DTYPE == "fp32":
        return {"f0t": f0t, "f1t": f1t}
    out = {}
    for name, x in (("f0", f0t), ("f1", f1t)):
        hi = x.astype(ml_dtypes.bfloat16)
        lo = (x - hi.astype(np.float32)).astype(ml_dtypes.bfloat16)
        out[name + "h"] = hi
        out[name + "l"] = lo
    return out


def _stage_inputs(feat_c0, feat_c1):
    import jax
    fn, feat_names, shardings = _get_runner()
    feats = [_feat_inputs(feat_c0, feat_c1, c) for c in range(8)]
    staged = []
    for name, sh in zip(feat_names, shardings):
        glob = np.concatenate([feats[c][name] for c in range(8)], axis=0)
        staged.append(jax.device_put(glob, sh))
    return staged


def kernel(feat_c0, feat_c1, h0c, w0c, h1c, w1c, h0i):
    feat_c0 = np.asarray(feat_c0, dtype=np.float32)
    feat_c1 = np.asarray(feat_c1, dtype=np.float32)
    h0c, w0c, h1c, w1c, h0i = int(h0c), int(w0c), int(h1c), int(w1c), int(h0i)

    fn, feat_names, shardings = _get_runner()
    staged = _stage_inputs(feat_c0, feat_c1)
    sim_g, conf_g = fn(staged)

    sim = np.asarray(sim_g).reshape(N, L, S)
    conf = np.asarray(conf_g).reshape(N, L, S)

    # ---- host post-processing (tiny outputs) ----
    rmax = conf.max(axis=2)                    # [N, L]
    jstar = conf.argmax(axis=2).astype(np.int64)
    cmax = conf.max(axis=1)                    # [N, S]

    valid_l = (_border_valid(h0c)[:, None] & _border_valid(w0c)[None, :]).reshape(-1)
    valid_s = (_border_valid(h1c)[:, None] & _border_valid(w1c)[None, :]).reshape(-1)

    match = (rmax > THR) & valid_l[None, :] & valid_s[jstar] \
        & (rmax == np.take_along_axis(cmax, jstar, axis=1))
    all_j_ids = np.where(match, jstar, 0).astype(np.int32)
    mask_v = match.astype(np.float32)
    mconf = (rmax * mask_v).astype(np.float32)

    scale = np.float32(h0i) / np.float32(h0c)
    i_ids = np.arange(h0c * w0c)
    mkpts0 = (np.stack([i_ids % w0c, i_ids // w0c], axis=1)
              .astype(np.float32) * scale)
    mkpts1 = (np.stack([all_j_ids % w1c, all_j_ids // w1c], axis=-1)
              .astype(np.float32) * scale)

    return conf, sim, mask_v, all_j_ids, mconf, mkpts0, mkpts1


def time_hw(inputs, k_small=2, k_big=12, reps=3):
    """Per-iteration device time of the A+glue+B pipeline.

    A single dispatch over the axon relay costs ~96 ms of round-trip latency,
    so wall-timing one call measures the network, not the hardware.  Instead
    we enqueue k back-to-back pipelines asynchronously and block once at the
    end; the slope between k_small and k_big iterations is the marginal
    per-pipeline device time (enqueued executions run back-to-back on the
    device side).
    """
    import time
    import jax
    feat_c0 = np.asarray(inputs["feat_c0"], np.float32)
    feat_c1 = np.asarray(inputs["feat_c1"], np.float32)
    fn, _, _ = _get_runner()
    staged = _stage_inputs(feat_c0, feat_c1)
    jax.block_until_ready(fn(staged))  # warm/compile

    def run_k(k):
        best = float("inf")
        for _ in range(reps):
            t0 = time.perf_counter()
            out = None
            for _ in range(k):
                out = fn(staged)
            jax.block_until_ready(out)
            best = min(best, time.perf_counter() - t0)
        return best

    t_small, t_big = run_k(k_small), run_k(k_big)
    return (t_big - t_small) / (k_big - k_small) * 1e9


# revision 25
# speedup vs baseline: 1.3586x; 1.3586x over previous
"""CoarseMatching (LoFTR-style dual-softmax matching) on 8 Trainium2 cores.

Sharding: core c handles batch n = c//2, L-half h = c%2 (rows [h*2400, (h+1)*2400)).
Each core computes its sim shard (matmul), conf shard (dual softmax), and the
host derives the small outputs (mutual-NN / threshold / keypoints) from conf.

conf[l,s] = exp(2*st - 2C) / (R'_l * S'_s)   with st = 10*sim, C = 45,
R'_l = sum_s exp(st - C), S'_s = sum_l exp(st - C)  (softmax shift cancels
exactly; C keeps exp() in fp32 range).

Matmul precision: "bf16x3" splits each f32 feature into hi=bf16(x) and
lo=bf16(x-hi); sim = hi.hi + hi.lo + lo.hi accumulated in f32 PSUM gives
~1e-5 relative accuracy at 3x bf16 rate (vs 4x slower native fp32).

Pipeline (3 device programs, data stays on-device between them):
  A: sim matmul -> sim out; E = exp(10*sim - C); row sums -> on-core
     rbias = -(2C + log R'); column sums tree-reduced to [1, S].
  glue: pure lax.psum of the [1, S] column-sum partials across the 2 cores
     of each batch (the only cross-core step).
  B: conf = exp(20*sim - (2C + log R')) * (1/S'); sim comes from a mix of
     HBM re-reads (first NREAD blocks) and matmul recompute (rest) to
     balance the DMA and TensorE pipes.
"""

import sys
import contextlib

import numpy as np
import ml_dtypes

sys.path.insert(0, "/opt/trn_rl_repo")

import concourse.bass as bass  # noqa: E402
import concourse.tile as tile  # noqa: E402
from concourse import bacc, mybir  # noqa: E402

F32 = mybir.dt.float32
BF16 = mybir.dt.bfloat16

# Problem geometry (hardcoded per contract)
N, L, S, C = 4, 4800, 4800, 256
LH = L // 2                  # 2400 rows per core
NBLK = (LH + 127) // 128     # 19 row blocks (last one 96 rows)
SCHUNK = 480                 # psum chunk (one bank holds 512 f32)
NCHUNK = S // SCHUNK         # 10
TEMP = 0.1
THR = 0.2
BORDER = 2
SHIFT = 45.0                 # constant softmax shift C

MM_DTYPE = "bf16x3"          # "fp32" | "bf16x3"
NREAD = 6                    # B: blocks whose sim is re-read from HBM
PSUM_BUFS = 4

_CACHE = {}


def _declare_feats(nc):
    if MM_DTYPE == "fp32":
        names = [("f0t", [C, LH], F32), ("f1t", [C, S], F32)]
    else:
        names = [("f0h", [C, LH], BF16), ("f0l", [C, LH], BF16),
                 ("f1h", [C, S], BF16), ("f1l", [C, S], BF16)]
    return [nc.dram_tensor(n, sh, dt, kind="ExternalInput").ap()
            for (n, sh, dt) in names]


def _load_feats(nc, pool, feat_aps):
    tiles = []
    for i, ap in enumerate(feat_aps):
        free = ap.shape[1]
        t = pool.tile([128, 2, free], ap.dtype, tag=f"feat{i}")
        nc.sync.dma_start(t[:], ap.rearrange("(c k) f -> k c f", k=128))
        tiles.append(t)
    if MM_DTYPE == "fp32":
        return [tiles[0]], [tiles[1]], [(0, 0)]
    return [tiles[0], tiles[1]], [tiles[2], tiles[3]], [(0, 0), (0, 1), (1, 0)]


def _emit_matmul(nc, ps, lhs_parts, rhs_parts, mm_groups, b, pb, sj):
    lb = bass.ds(b * 128, pb)
    n_terms = len(mm_groups) * 2
    t = 0
    for (li, ri) in mm_groups:
        for cc in range(2):
            t += 1
            nc.tensor.matmul(
                ps[:pb, :],
                lhs_parts[li][:, cc, lb],
                rhs_parts[ri][:, cc, sj],
                start=(t == 1),
                stop=(t == n_terms),
            )


def _build_a():
    nc = bacc.Bacc("TRN2", target_bir_lowering=False, debug=False)
    feat_aps = _declare_feats(nc)
    sim_out = nc.dram_tensor("sim", [LH, S], F32, kind="ExternalOutput").ap()
    colsum_out = nc.dram_tensor("colsum", [1, S], F32, kind="ExternalOutput").ap()
    rbias_out = nc.dram_tensor("rbias", [128, NBLK], F32, kind="ExternalOutput").ap()

    with tile.TileContext(nc) as tc, contextlib.ExitStack() as ctx:
        const_pool = ctx.enter_context(tc.tile_pool(name="const", bufs=1))
        psum = ctx.enter_context(tc.tile_pool(name="ps", bufs=PSUM_BUFS, space="PSUM"))
        simp = ctx.enter_context(tc.tile_pool(name="simp", bufs=3))
        ep = ctx.enter_context(tc.tile_pool(name="ep", bufs=2))
        accp = ctx.enter_context(tc.tile_pool(name="accp", bufs=1))

        lhs, rhs, mm_groups = _load_feats(nc, const_pool, feat_aps)

        acc = accp.tile([128, S], F32)
        nc.vector.memset(acc[:], 0.0)
        rstile = accp.tile([128, NBLK], F32)
        nc.vector.memset(rstile[:], 1.0)   # pad rows -> log(1) = 0
        nbias = accp.tile([128, 1], F32)
        nc.vector.memset(nbias[:], -SHIFT)

        for b in range(NBLK):
            pb = min(128, LH - b * 128)
            sim_sb = simp.tile([128, S], F32, tag="sim")
            e_sb = ep.tile([128, S], F32, tag="e")
            racc = ep.tile([128, NCHUNK], F32, tag="racc")
            for j in range(NCHUNK):
                ps = psum.tile([128, SCHUNK], F32, tag="mm")
                sj = bass.ts(j, SCHUNK)
                _emit_matmul(nc, ps, lhs, rhs, mm_groups, b, pb, sj)
                # raw sim copy (psum -> sbuf); nc.any lets Tile pick DVE/ACT
                nc.any.tensor_copy(sim_sb[:pb, sj], ps[:pb, :])
                # E = exp(10*sim - C), row-accumulated per chunk
                nc.scalar.activation(
                    e_sb[:pb, sj], ps[:pb, :],
                    mybir.ActivationFunctionType.Exp,
                    bias=nbias[:pb, 0:1], scale=10.0,
                    accum_out=racc[:pb, j:j + 1],
                )
            nc.sync.dma_start(sim_out[b * 128 : b * 128 + pb, :], sim_sb[:pb, :])
            nc.vector.tensor_add(acc[:pb, :], acc[:pb, :], e_sb[:pb, :])
            nc.vector.reduce_sum(rstile[:pb, b:b + 1], racc[:pb, :],
                                 axis=mybir.AxisListType.X)

        # column-sum partial: contract the 128 partitions with a ones-vector
        # matmul (DVE lanes cannot cross partitions)
        psum_red = ctx.enter_context(tc.tile_pool(name="psred", bufs=2, space="PSUM"))
        ones = accp.tile([128, 1], F32)
        nc.vector.memset(ones[:], 1.0)
        acc_red = accp.tile([1, S], F32)
        for j in range(NCHUNK):
            pr = psum_red.tile([1, SCHUNK], F32, tag="red")
            sj = bass.ts(j, SCHUNK)
            nc.tensor.matmul(pr[:, :], ones[:], acc[:, sj],
                             start=True, stop=True)
            nc.any.tensor_copy(acc_red[0:1, sj], pr[:, :])
        nc.sync.dma_start(colsum_out[:], acc_red[0:1, :])

        # rbias = -(2C + log R')
        logr = accp.tile([128, NBLK], F32)
        nc.scalar.activation(logr[:], rstile[:],
                             mybir.ActivationFunctionType.Ln)
        rbias_t = accp.tile([128, NBLK], F32)
        nc.scalar.activation(rbias_t[:], logr[:],
                             mybir.ActivationFunctionType.Copy,
                             bias=-2.0 * SHIFT, scale=-1.0)
        nc.sync.dma_start(rbias_out[:], rbias_t[:])

    nc.compile()
    return nc


def _build_b():
    nc = bacc.Bacc("TRN2", target_bir_lowering=False, debug=False)
    feat_aps = _declare_feats(nc)
    stot = nc.dram_tensor("stot", [1, S], F32, kind="ExternalInput").ap()
    rbias = nc.dram_tensor("rbias", [128, NBLK], F32, kind="ExternalInput").ap()
    sim_in = nc.dram_tensor("sim_in", [LH, S], F32, kind="ExternalInput").ap()
    conf_out = nc.dram_tensor("conf", [LH, S], F32, kind="ExternalOutput").ap()
    invs_dram = nc.dram_tensor("invs_bounce", [1, S], F32)

    with tile.TileContext(nc) as tc, contextlib.ExitStack() as ctx:
        const_pool = ctx.enter_context(tc.tile_pool(name="const", bufs=1))
        psum = ctx.enter_context(tc.tile_pool(name="ps", bufs=PSUM_BUFS, space="PSUM"))
        xp = ctx.enter_context(tc.tile_pool(name="xp", bufs=2))
        sinp = ctx.enter_context(tc.tile_pool(name="sinp", bufs=2))

        lhs, rhs, mm_groups = _load_feats(nc, const_pool, feat_aps)

        # invs = 1 / S'  broadcast to all 128 partitions via a DRAM bounce
        stot_sb = const_pool.tile([1, S], F32)
        nc.sync.dma_start(stot_sb[:], stot[:])
        nc.vector.reciprocal(stot_sb[:], stot_sb[:])
        nc.sync.dma_start(invs_dram[:], stot_sb[:])
        invs_b = const_pool.tile([128, S], F32)
        nc.sync.dma_start(invs_b[:], invs_dram[0:1, :].broadcast_to((128, S)))

        rbias_sb = const_pool.tile([128, NBLK], F32)
        nc.sync.dma_start(rbias_sb[:], rbias[:])

        for b in range(NBLK):
            pb = min(128, LH - b * 128)
            x_sb = xp.tile([128, S], F32, tag="x")
            if b < NREAD:
                sim_sb = sinp.tile([128, S], F32, tag="sin")
                nc.sync.dma_start(sim_sb[:pb, :],
                                  sim_in[b * 128 : b * 128 + pb, :])
                nc.scalar.activation(
                    x_sb[:pb, :], sim_sb[:pb, :],
                    mybir.ActivationFunctionType.Exp,
                    bias=rbias_sb[:pb, b:b + 1], scale=20.0,
                )
            else:
                for j in range(NCHUNK):
                    ps = psum.tile([128, SCHUNK], F32, tag="mm")
                    sj = bass.ts(j, SCHUNK)
                    _emit_matmul(nc, ps, lhs, rhs, mm_groups, b, pb, sj)
                    nc.scalar.activation(
                        x_sb[:pb, sj], ps[:pb, :],
                        mybir.ActivationFunctionType.Exp,
                        bias=rbias_sb[:pb, b:b + 1], scale=20.0,
                    )
            nc.vector.tensor_mul(x_sb[:pb, :], x_sb[:pb, :], invs_b[:pb, :])
            nc.sync.dma_start(conf_out[b * 128 : b * 128 + pb, :], x_sb[:pb, :])

    nc.compile()
    return nc


def _get_kernels():
    if "a" not in _CACHE:
        _CACHE["a"] = _build_a()
        _CACHE["b"] = _build_b()
    return _CACHE["a"], _CACHE["b"]


def _bir_spec(nc):
    """(input names, output names, output avals, partition name)."""
    import jax
    pname = nc.partition_id_tensor.name if nc.partition_id_tensor else None
    in_names, out_names, out_avals = [], [], []
    for alloc in nc.m.functions[0].allocations:
        if not isinstance(alloc, mybir.MemoryLocationSet):
            continue
        name = alloc.memorylocations[0].name
        if alloc.kind == "ExternalInput":
            if name != pname:
                in_names.append(name)
        elif alloc.kind == "ExternalOutput":
            shape = tuple(alloc.tensor_shape)
            dtype = mybir.dt.np(alloc.dtype)
            out_names.append(name)
            out_avals.append(jax.core.ShapedArray(shape, dtype))
    return in_names, out_names, out_avals, pname


def _bind_bass(nc, spec, operands):
    """operands: inputs in allocation order, then zero output buffers."""
    from concourse.bass2jax import _bass_exec_p, partition_id_tensor
    in_names, out_names, out_avals, pname = spec
    all_names = tuple(in_names) + tuple(out_names)
    operands = list(operands)
    if pname is not None:
        operands.append(partition_id_tensor())
        all_names = all_names + (pname,)
    outs = _bass_exec_p.bind(
        *operands,
        out_avals=tuple(out_avals),
        in_names=all_names,
        out_names=tuple(out_names),
        lowering_input_output_aliases=(),
        sim_require_finite=True,
        sim_require_nnan=True,
        nc=nc,
    )
    return dict(zip(out_names, outs))


def _get_runner():
    """Three jitted SPMD programs: launch A, psum glue, launch B."""
    if "runner" in _CACHE:
        return _CACHE["runner"]
    import jax
    import jax.numpy as jnp
    from jax.sharding import Mesh, PartitionSpec, NamedSharding
    from jax.experimental.shard_map import shard_map
    from concourse.bass2jax import install_neuronx_cc_hook

    install_neuronx_cc_hook()
    nc_a, nc_b = _get_kernels()
    spec_a, spec_b = _bir_spec(nc_a), _bir_spec(nc_b)

    devices = np.asarray(jax.devices()[:8]).reshape(4, 2)
    mesh = Mesh(devices, ("batch", "half"))
    pcore = PartitionSpec(("batch", "half"))
    shard = NamedSharding(mesh, pcore)

    feat_names = (["f0t", "f1t"] if MM_DTYPE == "fp32"
                  else ["f0h", "f0l", "f1h", "f1l"])
    nf = len(feat_names)
    assert list(spec_a[0]) == feat_names, spec_a[0]
    assert list(spec_b[0]) == feat_names + ["stot", "rbias", "sim_in"], spec_b[0]

    def body_a(*ops):
        outs = _bind_bass(nc_a, spec_a, ops)
        return outs["sim"], outs["colsum"], outs["rbias"]

    def body_glue(colsum):
        return jax.lax.psum(colsum, "half")

    def body_b(*ops):
        outs = _bind_bass(nc_b, spec_b, ops)
        return (outs["conf"],)

    n_a = nf + len(spec_a[1])
    n_b = nf + 3 + len(spec_b[1])
    fn_a = jax.jit(
        shard_map(body_a, mesh=mesh, in_specs=(pcore,) * n_a,
                  out_specs=(pcore,) * 3, check_rep=False))
    fn_glue = jax.jit(
        shard_map(body_glue, mesh=mesh, in_specs=(pcore,),
                  out_specs=pcore, check_rep=False))
    fn_b = jax.jit(
        shard_map(body_b, mesh=mesh, in_specs=(pcore,) * n_b,
                  out_specs=(pcore,), check_rep=False))

    def zeros_for(spec):
        _, _, out_avals, _ = spec
        return [jnp.zeros((8 * a.shape[0],) + tuple(a.shape[1:]), a.dtype)
                for a in out_avals]

    fn_zeros_a = jax.jit(lambda: tuple(zeros_for(spec_a)),
                         out_shardings=(shard,) * len(spec_a[1]))
    fn_zeros_b = jax.jit(lambda: tuple(zeros_for(spec_b)),
                         out_shardings=(shard,) * len(spec_b[1]))

    def run(staged_feats):
        if "zeros" not in _CACHE:
            _CACHE["zeros"] = (fn_zeros_a(), fn_zeros_b())
        za, zb = _CACHE["zeros"]
        sim, colsum, rbias = fn_a(*staged_feats, *za)
        stot = fn_glue(colsum)
        (conf,) = fn_b(*staged_feats, stot, rbias, sim, *zb)
        return sim, conf

    _CACHE["runner"] = (run, feat_names, [shard] * nf)
    return _CACHE["runner"]


def _border_valid(n):
    idx = np.arange(n)
    return (idx >= BORDER) & (idx < n - BORDER)


def _feat_inputs(feat_c0, feat_c1, core):
    # reference pre-scales each feature by 1/sqrt(C) = 1/16 (exact in fp32)
    n, h = core // 2, core % 2
    f0t = np.ascontiguousarray(feat_c0[n, h * LH : (h + 1) * LH].T) * np.float32(0.0625)
    f1t = np.ascontiguousarray(feat_c1[n].T) * np.float32(0.0625)
    if MM_DTYPE == "fp32":
        return {"f0t": f0t, "f1t": f1t}
    out = {}
    for name, x in (("f0", f0t), ("f1", f1t)):
        hi = x.astype(ml_dtypes.bfloat16)
        lo = (x - hi.astype(np.float32)).astype(ml_dtypes.bfloat16)
        out[name + "h"] = hi
        out[name + "l"] = lo
    return out


def _stage_inputs(feat_c0, feat_c1):
    import jax
    fn, feat_names, shardings = _get_runner()
    feats = [_feat_inputs(feat_c0, feat_c1, c) for c in range(8)]
    staged = []
    for name, sh in zip(feat_names, shardings):
        glob = np.concatenate([feats[c][name] for c in range(8)], axis=0)
        staged.append(jax.device_put(glob, sh))
    return staged


def kernel(feat_c0, feat_c1, h0c, w0c, h1c, w1c, h0i):
    feat_c0 = np.asarray(feat_c0, dtype=np.float32)
    feat_c1 = np.asarray(feat_c1, dtype=np.float32)
    h0c, w0c, h1c, w1c, h0i = int(h0c), int(w0c), int(h1c), int(w1c), int(h0i)

    fn, feat_names, shardings = _get_runner()
    staged = _stage_inputs(feat_c0, feat_c1)
    sim_g, conf_g = fn(staged)

    sim = np.asarray(sim_g).reshape(N, L, S)
    conf = np.asarray(conf_g).reshape(N, L, S)

    # ---- host post-processing (tiny outputs) ----
    rmax = conf.max(axis=2)                    # [N, L]
    jstar = conf.argmax(axis=2).astype(np.int64)
    cmax = conf.max(axis=1)                    # [N, S]

    valid_l = (_border_valid(h0c)[:, None] & _border_valid(w0c)[None, :]).reshape(-1)
    valid_s = (_border_valid(h1c)[:, None] & _border_valid(w1c)[None, :]).reshape(-1)

    match = (rmax > THR) & valid_l[None, :] & valid_s[jstar] \
        & (rmax == np.take_along_axis(cmax, jstar, axis=1))
    all_j_ids = np.where(match, jstar, 0).astype(np.int32)
    mask_v = match.astype(np.float32)
    mconf = (rmax * mask_v).astype(np.float32)

    scale = np.float32(h0i) / np.float32(h0c)
    i_ids = np.arange(h0c * w0c)
    mkpts0 = (np.stack([i_ids % w0c, i_ids // w0c], axis=1)
              .astype(np.float32) * scale)
    mkpts1 = (np.stack([all_j_ids % w1c, all_j_ids // w1c], axis=-1)
              .astype(np.float32) * scale)

    return conf, sim, mask_v, all_j_ids, mconf, mkpts0, mkpts1


def time_hw(inputs, k_small=2, k_big=12, reps=5):
    """Per-iteration device time of the A+glue+B pipeline.

    A single dispatch over the axon relay costs ~96 ms of round-trip latency,
    so wall-timing one call measures the network, not the hardware.  Instead
    we enqueue k back-to-back pipelines asynchronously and block once at the
    end; the slope between k_small and k_big iterations is the marginal
    per-pipeline device time (enqueued executions run back-to-back on the
    device side).
    """
    import time
    import jax
    feat_c0 = np.asarray(inputs["feat_c0"], np.float32)
    feat_c1 = np.asarray(inputs["feat_c1"], np.float32)
    fn, _, _ = _get_runner()
    staged = _stage_inputs(feat_c0, feat_c1)
    jax.block_until_ready(fn(staged))  # warm/compile

    def run_k(k):
        best = float("inf")
        for _ in range(reps):
            t0 = time.perf_counter()
            out = None
            for _ in range(k):
                out = fn(staged)
            jax.block_until_ready(out)
            best = min(best, time.perf_counter() - t0)
        return best

    t_small, t_big = run_k(k_small), run_k(k_big)
    return (t_big - t_small) / (k_big - k_small) * 1e9
